# revision 20
# baseline (speedup 1.0000x reference)
"""Multi-head attention forward on 8 TRN2 NeuronCores.

Problem: B=4, S=2048, D=1024, H=16, d_k=64, fp32 in/out, mask == all-ones
(per the input spec the mask is always ones, so masking is a no-op and is
skipped).

Sharding (data-parallel over batch x query-blocks, no collectives):
  core c -> batch b = c//2, query rows [ (c%2)*1024, (c%2)*1024+1024 ).
Each core computes the full forward for its 1024 query rows: Q/K/V
projections (K/V over all 2048 keys of its batch), attention, and the
output projection. The host pre-transposes inputs and casts to bf16 (pure
data movement); every FLOP runs on-device.

All matmul operands are bf16 (fp32 HIGH-mode matmuls run ~2x slower on HW
than bf16; psum accumulation stays fp32). Device algorithm per core:
  A) QhT[(h,dk), q]  = wq.T @ Q.T   (d_model on partitions)
  B) KhT[(h,dk), k]  = wk.T @ K.T
  C) Vh[k, (h,dk)]   = (V.T).T @ wv, with a ones column per head
  D) per head pair p (head 2p on partitions 0-63, head 2p+1 on 64-127):
     per key tile km: S^T[k, q] = KhT.T @ QhT; exp with the 1/8 scale
     folded in -- split between ACT (exact exp) and DVE (16-bit
     Schraudolph: int16(x*A16+B16) bitcast bf16) to keep both engines
     under the PE's critical path; attn_unnorm^T = [Vh | 1]^T @ exp(S^T)
     accumulated over km with the AV matmuls lagging exp by one key tile
     so the PE never waits on the activations.  PSUM row 64 of the AV
     accumulator is the softmax denominator l; the per-head tail (evict,
     1/l via a DRAM-reshape + [128,8] reciprocal, broadcast multiply) is
     deferred into the next pair's km loop so it never head-of-line
     blocks the DVE queue.
  E) out[q, d] = attnT.T @ wo + bias (accumulate over the pair axis).
"""

import os
import sys

for _p in ("/root/.axon_site/_ro/trn_rl_repo", "/opt/trn_rl_repo"):
    if os.path.isdir(_p) and _p not in sys.path:
        sys.path.append(_p)

import ml_dtypes
import numpy as np

import concourse.bass as bass  # noqa: F401  (import keeps bass_rust registered)
import concourse.tile as tile
from concourse import bacc, mybir
from concourse.bass_utils import run_bass_kernel_spmd

P = 128
D = 1024  # d_model
S = 2048  # sequence length (keys per batch)
QL = 1024  # query rows per core
H = 16
DK = 64
NPAIR = H // 2  # pair p holds head 2p on partitions 0-63, head 2p+1 on 64-127
DKT = D // P  # 8 contraction tiles over d_model
KMT = S // P  # 16 key-row tiles
QMT = QL // P  # 8 query-row tiles
S2 = S // 2  # keys owned per core before the pair exchange
F32 = mybir.dt.float32
F32R = mybir.dt.float32r
BF16 = mybir.dt.bfloat16
I16 = mybir.dt.int16
EXP = mybir.ActivationFunctionType.Exp
MULT = mybir.AluOpType.mult
ADD = mybir.AluOpType.add
# 16-bit Schraudolph exp(x/8): int16(x*A16 + B16) bitcast to bf16.
# Used on DVE for a subset of score tiles to offload the ACT engine.
A16 = 0.125 * 1.4426950408889634 * 128.0
B16 = 1064866805.0 / 65536.0
# (km, head) indices routed to the DVE Schraudolph exp: idx = 2*km + head,
# DVE when idx % 8 in this set -> 8 of 32 tiles per pair on DVE.
DVE_EXP = {1, 4}

LAST_RESULTS = None  # test harness reads exec_time_ns from here


def _r(ap):
    """Reinterpret an fp32 AP as float32r (truncated matmul dtype)."""
    return ap.bitcast(F32R)


def _build_nc():
    nc = bacc.Bacc("TRN2", debug=False, target_bir_lowering=False,
                   num_devices=8)

    qt = nc.dram_tensor("qt", [D, QL], BF16, kind="ExternalInput").ap()
    ktd = nc.dram_tensor("ktd", [D, S2], BF16, kind="ExternalInput").ap()
    vtd = nc.dram_tensor("vtd", [D, S2], BF16, kind="ExternalInput").ap()
    wq = nc.dram_tensor("wq", [D, D], BF16, kind="ExternalInput").ap()
    wk = nc.dram_tensor("wk", [D, D], BF16, kind="ExternalInput").ap()
    wv = nc.dram_tensor("wv", [D, D], BF16, kind="ExternalInput").ap()
    wo = nc.dram_tensor("wo", [D, D], BF16, kind="ExternalInput").ap()
    wob = nc.dram_tensor("wob", [1, D], F32, kind="ExternalInput").ap()
    out = nc.dram_tensor("out", [QL, D], F32, kind="ExternalOutput").ap()
    # DRAM scratch for the K/V-projection exchange between paired cores:
    # each core projects its own 1024 keys, AllGathers with its partner
    # (groups of 2: rank r of [2b, 2b+1] owns keys r*1024:(r+1)*1024)
    khx_i = nc.dram_tensor("khx_i", [P, NPAIR, S2], BF16, kind="Internal").ap()
    khx_o = nc.dram_tensor("khx_o", [2, P, NPAIR, S2], BF16, kind="Internal").ap()
    vhx_i = nc.dram_tensor("vhx_i", [P, KMT // 2, H, 66], BF16, kind="Internal").ap()
    vhx_o = nc.dram_tensor("vhx_o", [2, P, KMT // 2, H, 66], BF16, kind="Internal").ap()
    # DRAM scratch for the softmax-denominator reshape/broadcast bounce
    ld = nc.dram_tensor("ld", [H, QL], F32, kind="Internal").ap()
    rcd = nc.dram_tensor("rcd", [H, QL], F32, kind="Internal").ap()
    # row h viewed as [128 partitions, 8 chunks]: element (p, c) at c*128+p
    ld3 = ld.rearrange("h (c p) -> h p c", p=P)

    qt3 = qt.rearrange("(kt p) q -> p kt q", p=P)  # [128, 8, 1024]
    kt3 = ktd.rearrange("(kt p) s -> p kt s", p=P)  # [128, 8, 2048]
    vt3 = vtd.rearrange("(kt p) s -> p kt s", p=P)
    wq3 = wq.rearrange("(kt p) e -> p kt e", p=P)
    wk3 = wk.rearrange("(kt p) e -> p kt e", p=P)
    wv3 = wv.rearrange("(kt p) e -> p kt e", p=P)
    wo3 = wo.rearrange("(kt p) e -> p kt e", p=P)
    out3 = out.rearrange("(mt p) e -> p mt e", p=P)

    with tile.TileContext(nc) as tc:
        mm = nc.tensor.matmul

        # ------- persistent SBUF -------
        pers = tc.alloc_tile_pool(name="pers", bufs=1)
        qh = pers.tile([P, NPAIR, QL], BF16)  # QhT: pair partitions x pair x q
        kh = pers.tile([P, NPAIR, S], BF16)  # KhT
        vh = pers.tile([P, KMT, H, 66], BF16)  # keys x km x head x (dk|1|pad)
        attn = pers.tile([P, NPAIR, QL], BF16)  # attn^T (normalized)
        bias_sb = pers.tile([P, D], F32)

        nc.vector.memset(vh[:, :, :, 64:66], 1.0)

        # bias broadcast [1,D] -> [128,D] via 0-stride-partition DMA read
        wob_bcast = bass.AP(tensor=wob.tensor, offset=wob.offset,
                            ap=[[0, P]] + [list(d) for d in wob.ap[1:]])
        nc.gpsimd.dma_start(out=bias_sb, in_=wob_bcast)

        # ------- staging -------
        pKt = tc.alloc_tile_pool(name="pKt", bufs=1, side="right")
        pQt = tc.alloc_tile_pool(name="pQt", bufs=1, side="right")
        pSm = tc.alloc_tile_pool(name="pSm", bufs=4)  # wq/vt tiles
        pWk = tc.alloc_tile_pool(name="pWk", bufs=2)  # wk tiles

        kt_sb = pKt.tile([P, DKT, S2], BF16)
        qt_sb = pQt.tile([P, DKT, QL], BF16)
        # first weight tiles DMA'd before the staging chunks so phase B's
        # first matmuls aren't FIFO-blocked behind the staging traffic
        wk_pre = []
        for mt in range(4):
            wk_t = pWk.tile([P, DKT, P], BF16, tag="wk")
            nc.sync.dma_start(out=wk_t, in_=wk3[:, :, mt * P : (mt + 1) * P])
            wk_pre.append(wk_t)
        for k in range(DKT):  # chunked: first matmuls start early
            nc.sync.dma_start(out=kt_sb[:, k, :], in_=kt3[:, k, :])

        psum_pr = tc.alloc_tile_pool(name="psum_pr", bufs=8, space="PSUM")

        # -------- phase B': K projection, own 1024 keys only --------
        for mt in range(DKT):
            if mt < 4:
                wk_t = wk_pre[mt]
            else:
                wk_t = pWk.tile([P, DKT, P], BF16, tag="wk")
                nc.sync.dma_start(out=wk_t, in_=wk3[:, :, mt * P : (mt + 1) * P])
            ps0 = psum_pr.tile([P, 512], F32, tag="prps")
            ps1 = psum_pr.tile([P, 512], F32, tag="prps")
            for k in range(DKT):
                st, sp = k == 0, k == DKT - 1
                mm(ps0, wk_t[:, k, :], kt_sb[:, k, 0:512],
                   start=st, stop=sp, skip_group_check=True)
                mm(ps1, wk_t[:, k, :], kt_sb[:, k, 512:1024],
                   start=st, stop=sp, skip_group_check=True)
            nc.vector.tensor_copy(out=kh[:, mt, 0:512], in_=ps0)
            nc.vector.tensor_copy(out=kh[:, mt, 512:1024], in_=ps1)

        # kh exchange (overlaps C' and A): own half -> DRAM -> AllGather
        # with the partner core -> both halves back into kh.  Exchange DMAs
        # ride the otherwise-idle gpsimd queue so they never FIFO-block the
        # sync queue's staging traffic; the collective runs on gpsimd.
        GROUPS = [[0, 1], [2, 3], [4, 5], [6, 7]]
        nc.gpsimd.dma_start(out=khx_i, in_=kh[:, :, 0:S2])
        nc.gpsimd.collective_compute(
            "AllGather", mybir.AluOpType.bypass, replica_groups=GROUPS,
            ins=[khx_i], outs=[khx_o])
        nc.gpsimd.dma_start(out=kh[:, :, 0:S2], in_=khx_o[0])
        nc.gpsimd.dma_start(out=kh[:, :, S2:S], in_=khx_o[1])

        # wv DMA then C' staging
        pWv = tc.alloc_tile_pool(name="pWv", bufs=1)
        wv_sb = pWv.tile([P, DKT, D], BF16)
        nc.sync.dma_start(out=wv_sb, in_=wv3)

        # -------- phase C': V projection, own 1024 keys only --------
        for km in range(KMT // 2):
            vt_t = pSm.tile([P, DKT, P], BF16, tag="sm")
            nc.sync.dma_start(out=vt_t, in_=vt3[:, :, km * P : (km + 1) * P])
            ps0 = psum_pr.tile([P, 512], F32, tag="prps")
            ps1 = psum_pr.tile([P, 512], F32, tag="prps")
            for k in range(DKT):
                st, sp = k == 0, k == DKT - 1
                mm(ps0, vt_t[:, k, :], wv_sb[:, k, 0:512],
                   start=st, stop=sp, skip_group_check=True)
                mm(ps1, vt_t[:, k, :], wv_sb[:, k, 512:1024],
                   start=st, stop=sp, skip_group_check=True)
            nc.scalar.copy(
                out=vh[:, km, 0:8, 0:64],
                in_=ps0.rearrange("p (h e) -> p h e", e=DK),
            )
            nc.scalar.copy(
                out=vh[:, km, 8:16, 0:64],
                in_=ps1.rearrange("p (h e) -> p h e", e=DK),
            )

        # vh exchange (overlaps A); includes the memset ones columns
        nc.gpsimd.dma_start(out=vhx_i, in_=vh[:, 0 : KMT // 2, :, :])
        nc.gpsimd.collective_compute(
            "AllGather", mybir.AluOpType.bypass, replica_groups=GROUPS,
            ins=[vhx_i], outs=[vhx_o])
        nc.gpsimd.dma_start(out=vh[:, 0 : KMT // 2, :, :], in_=vhx_o[0])
        nc.gpsimd.dma_start(out=vh[:, KMT // 2 : KMT, :, :], in_=vhx_o[1])

        # ---------------- phase A: Q projection ----------------
        wq_pre = []
        for mt in range(2):
            wq_t = pSm.tile([P, DKT, P], BF16, tag="sm")
            nc.sync.dma_start(out=wq_t, in_=wq3[:, :, mt * P : (mt + 1) * P])
            wq_pre.append(wq_t)
        for k in range(DKT):
            nc.sync.dma_start(out=qt_sb[:, k, :], in_=qt3[:, k, :])
        for mt in range(DKT):
            if mt < 2:
                wq_t = wq_pre[mt]
            else:
                wq_t = pSm.tile([P, DKT, P], BF16, tag="sm")
                nc.sync.dma_start(out=wq_t, in_=wq3[:, :, mt * P : (mt + 1) * P])
            ps0 = psum_pr.tile([P, 512], F32, tag="prps")
            ps1 = psum_pr.tile([P, 512], F32, tag="prps")
            for k in range(DKT):
                st, sp = k == 0, k == DKT - 1
                mm(ps0, wq_t[:, k, :], qt_sb[:, k, 0:512],
                   start=st, stop=sp, skip_group_check=True)
                mm(ps1, wq_t[:, k, :], qt_sb[:, k, 512:1024],
                   start=st, stop=sp, skip_group_check=True)
            nc.vector.tensor_copy(out=qh[:, mt, 0:512], in_=ps0)
            nc.vector.tensor_copy(out=qh[:, mt, 512:1024], in_=ps1)
        pQt.release()
        pKt.release()
        pWv.release()
        pWk.release()
        pSm.release()
        psum_pr.release()

        # wo DMA overlaps phase D
        pWo = tc.alloc_tile_pool(name="pWo", bufs=1)
        wo_sb = pWo.tile([P, DKT, D], BF16)
        for k in range(DKT):
            nc.sync.dma_start(out=wo_sb[:, k, :], in_=wo3[:, k, :])

        # ---------------- phase D: attention per head pair ----------------
        psum_av = tc.alloc_tile_pool(name="psum_av", bufs=2, space="PSUM")
        psum_st = tc.alloc_tile_pool(name="psum_st", bufs=2, space="PSUM")
        pEx = tc.alloc_tile_pool(name="pEx", bufs=4)
        pRc = tc.alloc_tile_pool(name="pRc", bufs=4)

        # Softmax tail note: the reciprocal and final multiply wait on DMA
        # round trips, so they are DEFERRED into the middle of the NEXT
        # pair's km loop -- otherwise they head-of-line block the DVE queue
        # (the next pair's exp tiles queue behind them), which transitively
        # stalls the PE and drops it out of its top p-state.
        prev_rc, prev_mul = [], []

        for p in range(NPAIR):
            hA, hB = 2 * p, 2 * p + 1
            avA = psum_av.tile([65, QL], F32, tag="av")
            avB = psum_av.tile([65, QL], F32, tag="av")

            def emit_av(k, exA, exB, avA=avA, avB=avB, hA=hA, hB=hB):
                stF, spF = k == 0, k == KMT - 1
                mm(avA[:, 0:512], vh[:, k, hA, 0:65], exA[:, 0:512],
                   start=stF, stop=spF, skip_group_check=True)
                mm(avA[:, 512:1024], vh[:, k, hA, 0:65], exA[:, 512:1024],
                   start=stF, stop=spF, skip_group_check=True)
                mm(avB[:, 0:512], vh[:, k, hB, 0:65], exB[:, 0:512],
                   start=stF, stop=spF, skip_group_check=True)
                mm(avB[:, 512:1024], vh[:, k, hB, 0:65], exB[:, 512:1024],
                   start=stF, stop=spF, skip_group_check=True)

            def emit_exp(st_ps, ex_t, idx):
                if idx % 8 in DVE_EXP:
                    nc.vector.tensor_scalar(
                        out=ex_t.bitcast(I16), in0=st_ps,
                        scalar1=A16, scalar2=B16, op0=MULT, op1=ADD)
                else:
                    nc.scalar.activation(ex_t, st_ps, EXP, scale=0.125)

            prev = None
            for km in range(KMT):
                if km == 5:
                    for f in prev_rc:
                        f()
                    prev_rc = []
                if km == 11:
                    for f in prev_mul:
                        f()
                    prev_mul = []
                kslA = kh[0:64, p, km * P : (km + 1) * P]
                kslB = kh[64:128, p, km * P : (km + 1) * P]
                stA = psum_st.tile([P, QL], F32, tag="st")
                stB = psum_st.tile([P, QL], F32, tag="st")
                mm(stA[:, 0:512], kslA, qh[0:64, p, 0:512])
                mm(stA[:, 512:1024], kslA, qh[0:64, p, 512:1024])
                mm(stB[:, 0:512], kslB, qh[64:128, p, 0:512])
                mm(stB[:, 512:1024], kslB, qh[64:128, p, 512:1024])
                exA = pEx.tile([P, QL], BF16, tag="ex")
                exB = pEx.tile([P, QL], BF16, tag="ex")
                emit_exp(stA, exA, 2 * km)
                emit_exp(stB, exB, 2 * km + 1)
                if prev is not None:
                    emit_av(km - 1, *prev)
                prev = (exA, exB)
            emit_av(KMT - 1, *prev)

            # tail stage 1 (now): rows 0-63 = attn_unnorm^T, row 64 = l.
            # Evict both immediately so the av psum frees; ship l to DRAM
            # and reload it reshaped [128, 8] (a [1, N] DVE reciprocal is
            # serial at ~6.4ns/elem; the reshaped one is ~60x faster).
            for head, av in ((0, avA), (1, avB)):
                rows = slice(0, 64) if head == 0 else slice(64, 128)
                h = 2 * p + head
                lrow = pRc.tile([1, QL], F32, tag="lrow")
                nc.vector.tensor_copy(out=lrow, in_=av[64:65, :])
                nc.vector.tensor_copy(out=attn[rows, p, :], in_=av[0:64, :])
                nc.sync.dma_start(out=ld[h : h + 1, :], in_=lrow)
                l128 = pRc.tile([P, QMT], F32, tag="l128")
                nc.sync.dma_start(out=l128, in_=ld3[h])
                rc_holder = []

                def run_rc(h=h, l128=l128, rc_holder=rc_holder):
                    rc128 = pRc.tile([P, QMT], F32, tag="rc128")
                    nc.vector.reciprocal(rc128, l128)
                    rcrow = rcd[h : h + 1, :]
                    rc3 = rcrow.rearrange("o (c p) -> (o p) c", p=P)
                    nc.sync.dma_start(out=rc3, in_=rc128)
                    rc_bc = pRc.tile([P, QL], F32, tag="rcbc")
                    rc_bcast = bass.AP(
                        tensor=rcrow.tensor, offset=rcrow.offset,
                        ap=[[0, P]] + [list(x) for x in rcrow.ap[1:]])
                    nc.sync.dma_start(out=rc_bc, in_=rc_bcast)
                    rc_holder.append(rc_bc)

                def run_mul(rows=rows, p=p, rc_holder=rc_holder):
                    nc.vector.tensor_mul(attn[rows, p, :], attn[rows, p, :],
                                         rc_holder[0][rows, :])

                prev_rc.append(run_rc)
                prev_mul.append(run_mul)

        # last pair's tail has no following pair to hide in
        for f in prev_rc:
            f()
        for f in prev_mul:
            f()
        pRc.release()
        pEx.release()
        psum_st.release()
        psum_av.release()

        # ---------------- phase E: output projection ----------------
        psum_E = tc.alloc_tile_pool(name="psum_E", bufs=4, space="PSUM")
        pOut = tc.alloc_tile_pool(name="pOut", bufs=2)

        for mt in range(QMT):
            ps0 = psum_E.tile([P, 512], F32, tag="eps")
            ps1 = psum_E.tile([P, 512], F32, tag="eps")
            for k in range(DKT):
                st, sp = k == 0, k == DKT - 1
                a_sl = attn[:, k, mt * P : (mt + 1) * P]
                mm(ps0, a_sl, wo_sb[:, k, 0:512],
                   start=st, stop=sp, skip_group_check=True)
                mm(ps1, a_sl, wo_sb[:, k, 512:1024],
                   start=st, stop=sp, skip_group_check=True)
            o_sb = pOut.tile([P, D], F32, tag="osb")
            nc.vector.tensor_add(out=o_sb[:, 0:512], in0=ps0,
                                 in1=bias_sb[:, 0:512])
            nc.vector.tensor_add(out=o_sb[:, 512:1024], in0=ps1,
                                 in1=bias_sb[:, 512:1024])
            nc.sync.dma_start(out=out3[:, mt, :], in_=o_sb)

        pOut.release()
        psum_E.release()
        pWo.release()
        pers.release()

    nc.compile()
    return nc


_NC = None


def _get_nc():
    global _NC
    if _NC is None:
        _NC = _build_nc()
    return _NC


def kernel(Q, K, V, mask, W_q, W_k, W_v, W_o_w, W_o_b):
    global LAST_RESULTS
    BF = ml_dtypes.bfloat16
    Q = np.asarray(Q, dtype=np.float32)
    K = np.asarray(K, dtype=np.float32)
    V = np.asarray(V, dtype=np.float32)
    W_q = np.asarray(W_q, dtype=np.float32)
    W_k = np.asarray(W_k, dtype=np.float32)
    W_v = np.asarray(W_v, dtype=np.float32)
    W_o_w = np.asarray(W_o_w, dtype=np.float32)
    W_o_b = np.asarray(W_o_b, dtype=np.float32)

    # weight shards (shared by all cores); host-side transpose + bf16 cast is
    # data movement only
    wq_h = np.ascontiguousarray(W_q.transpose(1, 0, 2).reshape(D, D).astype(BF))
    wk_h = np.ascontiguousarray(W_k.transpose(1, 0, 2).reshape(D, D).astype(BF))
    wv_h = np.ascontiguousarray(W_v.transpose(1, 0, 2).reshape(D, D).astype(BF))
    wo_h = np.ascontiguousarray(W_o_w.T.astype(BF))
    wob_h = np.ascontiguousarray(W_o_b.reshape(1, D))

    in_maps = []
    for c in range(8):
        b, qs = c // 2, (c % 2) * QL
        in_maps.append({
            "qt": np.ascontiguousarray(Q[b, qs : qs + QL, :].T.astype(BF)),
            "ktd": np.ascontiguousarray(K[b].T[:, qs : qs + QL].astype(BF)),
            "vtd": np.ascontiguousarray(V[b].T[:, qs : qs + QL].astype(BF)),
            "wq": wq_h,
            "wk": wk_h,
            "wv": wv_h,
            "wo": wo_h,
            "wob": wob_h,
        })

    nc = _get_nc()
    res = run_bass_kernel_spmd(nc, in_maps, core_ids=list(range(8)))
    LAST_RESULTS = res

    out = np.empty((4, 2 * QL, D), dtype=np.float32)
    for c in range(8):
        b, qs = c // 2, (c % 2) * QL
        out[b, qs : qs + QL, :] = res.results[c]["out"]
    return out


# revision 21
# speedup vs baseline: 1.4064x; 1.4064x over previous
"""Multi-head attention forward on 8 TRN2 NeuronCores.

Problem: B=4, S=2048, D=1024, H=16, d_k=64, fp32 in/out, mask == all-ones
(per the input spec the mask is always ones, so masking is a no-op and is
skipped).

Sharding (data-parallel over batch x query-blocks, no collectives):
  core c -> batch b = c//2, query rows [ (c%2)*1024, (c%2)*1024+1024 ).
Each core computes the full forward for its 1024 query rows: Q/K/V
projections (K/V over all 2048 keys of its batch), attention, and the
output projection. The host pre-transposes inputs and casts to bf16 (pure
data movement); every FLOP runs on-device.

All matmul operands are bf16 (fp32 HIGH-mode matmuls run ~2x slower on HW
than bf16; psum accumulation stays fp32). Device algorithm per core:
  A) QhT[(h,dk), q]  = wq.T @ Q.T   (d_model on partitions)
  B) KhT[(h,dk), k]  = wk.T @ K.T
  C) Vh[k, (h,dk)]   = (V.T).T @ wv, with a ones column per head
  D) per head pair p (head 2p on partitions 0-63, head 2p+1 on 64-127):
     per key tile km: S^T[k, q] = KhT.T @ QhT; exp with the 1/8 scale
     folded in -- split between ACT (exact exp) and DVE (16-bit
     Schraudolph: int16(x*A16+B16) bitcast bf16) to keep both engines
     under the PE's critical path; attn_unnorm^T = [Vh | 1]^T @ exp(S^T)
     accumulated over km with the AV matmuls lagging exp by one key tile
     so the PE never waits on the activations.  PSUM row 64 of the AV
     accumulator is the softmax denominator l; the per-head tail (evict,
     1/l via a DRAM-reshape + [128,8] reciprocal, broadcast multiply) is
     deferred into the next pair's km loop so it never head-of-line
     blocks the DVE queue.
  E) out[q, d] = attnT.T @ wo + bias (accumulate over the pair axis).
"""

import os
import sys

for _p in ("/root/.axon_site/_ro/trn_rl_repo", "/opt/trn_rl_repo"):
    if os.path.isdir(_p) and _p not in sys.path:
        sys.path.append(_p)

import ml_dtypes
import numpy as np

import concourse.bass as bass  # noqa: F401  (import keeps bass_rust registered)
import concourse.tile as tile
from concourse import bacc, mybir
from concourse.bass_utils import run_bass_kernel_spmd

P = 128
D = 1024  # d_model
S = 2048  # sequence length (keys per batch)
QL = 1024  # query rows per core
H = 16
DK = 64
NPAIR = H // 2  # pair p holds head 2p on partitions 0-63, head 2p+1 on 64-127
DKT = D // P  # 8 contraction tiles over d_model
KMT = S // P  # 16 key-row tiles
QMT = QL // P  # 8 query-row tiles
F32 = mybir.dt.float32
F32R = mybir.dt.float32r
BF16 = mybir.dt.bfloat16
I16 = mybir.dt.int16
EXP = mybir.ActivationFunctionType.Exp
MULT = mybir.AluOpType.mult
ADD = mybir.AluOpType.add
# 16-bit Schraudolph exp(x/8): int16(x*A16 + B16) bitcast to bf16.
# Used on DVE for a subset of score tiles to offload the ACT engine.
A16 = 0.125 * 1.4426950408889634 * 128.0
B16 = 1064866805.0 / 65536.0
# (km, head) indices routed to the DVE Schraudolph exp: idx = 2*km + head,
# DVE when idx % 8 in this set -> 8 of 32 tiles per pair on DVE.
DVE_EXP = {1, 4}

LAST_RESULTS = None  # test harness reads exec_time_ns from here


def _r(ap):
    """Reinterpret an fp32 AP as float32r (truncated matmul dtype)."""
    return ap.bitcast(F32R)


def _build_nc():
    nc = bacc.Bacc("TRN2", debug=False, target_bir_lowering=False)

    qt = nc.dram_tensor("qt", [D, QL], BF16, kind="ExternalInput").ap()
    ktd = nc.dram_tensor("ktd", [D, S], BF16, kind="ExternalInput").ap()
    vtd = nc.dram_tensor("vtd", [D, S], BF16, kind="ExternalInput").ap()
    wq = nc.dram_tensor("wq", [D, D], BF16, kind="ExternalInput").ap()
    wk = nc.dram_tensor("wk", [D, D], BF16, kind="ExternalInput").ap()
    wv = nc.dram_tensor("wv", [D, D], BF16, kind="ExternalInput").ap()
    wo = nc.dram_tensor("wo", [D, D], BF16, kind="ExternalInput").ap()
    wob = nc.dram_tensor("wob", [1, D], F32, kind="ExternalInput").ap()
    out = nc.dram_tensor("out", [QL, D], F32, kind="ExternalOutput").ap()
    # DRAM scratch for the softmax-denominator reshape/broadcast bounce
    ld = nc.dram_tensor("ld", [H, QL], F32, kind="Internal").ap()
    rcd = nc.dram_tensor("rcd", [H, QL], F32, kind="Internal").ap()
    # row h viewed as [128 partitions, 8 chunks]: element (p, c) at c*128+p
    ld3 = ld.rearrange("h (c p) -> h p c", p=P)

    qt3 = qt.rearrange("(kt p) q -> p kt q", p=P)  # [128, 8, 1024]
    kt3 = ktd.rearrange("(kt p) s -> p kt s", p=P)  # [128, 8, 2048]
    vt3 = vtd.rearrange("(kt p) s -> p kt s", p=P)
    wq3 = wq.rearrange("(kt p) e -> p kt e", p=P)
    wk3 = wk.rearrange("(kt p) e -> p kt e", p=P)
    wv3 = wv.rearrange("(kt p) e -> p kt e", p=P)
    wo3 = wo.rearrange("(kt p) e -> p kt e", p=P)
    out3 = out.rearrange("(mt p) e -> p mt e", p=P)

    with tile.TileContext(nc) as tc:
        mm = nc.tensor.matmul

        # ------- persistent SBUF -------
        pers = tc.alloc_tile_pool(name="pers", bufs=1)
        qh = pers.tile([P, NPAIR, QL], BF16)  # QhT: pair partitions x pair x q
        kh = pers.tile([P, NPAIR, S], BF16)  # KhT
        vh = pers.tile([P, KMT, H, 66], BF16)  # keys x km x head x (dk|1|pad)
        attn = pers.tile([P, NPAIR, QL], BF16)  # attn^T (normalized)
        bias_sb = pers.tile([P, D], F32)

        nc.vector.memset(vh[:, :, :, 64:65], 1.0)

        # bias broadcast [1,D] -> [128,D] via 0-stride-partition DMA read
        wob_bcast = bass.AP(tensor=wob.tensor, offset=wob.offset,
                            ap=[[0, P]] + [list(d) for d in wob.ap[1:]])
        nc.gpsimd.dma_start(out=bias_sb, in_=wob_bcast)

        # ------- staging -------
        pKt = tc.alloc_tile_pool(name="pKt", bufs=1, side="right")
        pQt = tc.alloc_tile_pool(name="pQt", bufs=1, side="right")
        pSm = tc.alloc_tile_pool(name="pSm", bufs=4)  # wq/vt tiles
        pWk = tc.alloc_tile_pool(name="pWk", bufs=2)  # wk tiles

        kt_sb = pKt.tile([P, DKT, S], BF16)
        qt_sb = pQt.tile([P, DKT, QL], BF16)
        # first weight tiles DMA'd before the staging chunks so phase A's
        # first matmuls aren't FIFO-blocked behind the staging traffic
        wq_pre = []
        for mt in range(4):
            wq_t = pSm.tile([P, DKT, P], BF16, tag="sm")
            nc.sync.dma_start(out=wq_t, in_=wq3[:, :, mt * P : (mt + 1) * P])
            wq_pre.append(wq_t)
        for k in range(DKT):  # chunked: first matmuls start early
            nc.sync.dma_start(out=qt_sb[:, k, :], in_=qt3[:, k, :])
        for k in range(DKT):
            nc.sync.dma_start(out=kt_sb[:, k, :], in_=kt3[:, k, :])

        psum_pr = tc.alloc_tile_pool(name="psum_pr", bufs=8, space="PSUM")

        # ---------------- phase A: Q projection ----------------
        for mt in range(DKT):
            if mt < 4:
                wq_t = wq_pre[mt]
            else:
                wq_t = pSm.tile([P, DKT, P], BF16, tag="sm")
                nc.sync.dma_start(out=wq_t, in_=wq3[:, :, mt * P : (mt + 1) * P])
            ps0 = psum_pr.tile([P, 512], F32, tag="prps")
            ps1 = psum_pr.tile([P, 512], F32, tag="prps")
            for k in range(DKT):
                st, sp = k == 0, k == DKT - 1
                mm(ps0, wq_t[:, k, :], qt_sb[:, k, 0:512],
                   start=st, stop=sp, skip_group_check=True)
                mm(ps1, wq_t[:, k, :], qt_sb[:, k, 512:1024],
                   start=st, stop=sp, skip_group_check=True)
            nc.vector.tensor_copy(out=qh[:, mt, 0:512], in_=ps0)
            nc.vector.tensor_copy(out=qh[:, mt, 512:1024], in_=ps1)
        pQt.release()

        # wv DMA overlaps phase B compute
        pWv = tc.alloc_tile_pool(name="pWv", bufs=1)
        wv_sb = pWv.tile([P, DKT, D], BF16)
        nc.sync.dma_start(out=wv_sb, in_=wv3)

        # ---------------- phase B: K projection ----------------
        for half in (0, 1):
            for mt in range(DKT):
                wk_t = pWk.tile([P, DKT, P], BF16, tag="wk")
                nc.sync.dma_start(out=wk_t, in_=wk3[:, :, mt * P : (mt + 1) * P])
                ps0 = psum_pr.tile([P, 512], F32, tag="prps")
                ps1 = psum_pr.tile([P, 512], F32, tag="prps")
                base = half * (S // 2)
                for k in range(DKT):
                    st, sp = k == 0, k == DKT - 1
                    mm(ps0, wk_t[:, k, :], kt_sb[:, k, base : base + 512],
                       start=st, stop=sp, skip_group_check=True)
                    mm(ps1, wk_t[:, k, :], kt_sb[:, k, base + 512 : base + 1024],
                       start=st, stop=sp, skip_group_check=True)
                nc.vector.tensor_copy(out=kh[:, mt, base : base + 512], in_=ps0)
                nc.vector.tensor_copy(out=kh[:, mt, base + 512 : base + 1024],
                                      in_=ps1)

        # ---------------- phase C: V projection ----------------
        for km in range(KMT):
            vt_t = pSm.tile([P, DKT, P], BF16, tag="sm")
            nc.sync.dma_start(out=vt_t, in_=vt3[:, :, km * P : (km + 1) * P])
            ps0 = psum_pr.tile([P, 512], F32, tag="prps")
            ps1 = psum_pr.tile([P, 512], F32, tag="prps")
            for k in range(DKT):
                st, sp = k == 0, k == DKT - 1
                mm(ps0, vt_t[:, k, :], wv_sb[:, k, 0:512],
                   start=st, stop=sp, skip_group_check=True)
                mm(ps1, vt_t[:, k, :], wv_sb[:, k, 512:1024],
                   start=st, stop=sp, skip_group_check=True)
            # C evictions on ACT (Copy) to keep DVE free for B's evictions
            nc.scalar.copy(
                out=vh[:, km, 0:8, 0:64],
                in_=ps0.rearrange("p (h e) -> p h e", e=DK),
            )
            nc.scalar.copy(
                out=vh[:, km, 8:16, 0:64],
                in_=ps1.rearrange("p (h e) -> p h e", e=DK),
            )
        pKt.release()
        pWv.release()
        pWk.release()
        pSm.release()
        psum_pr.release()

        # wo DMA overlaps phase D
        pWo = tc.alloc_tile_pool(name="pWo", bufs=1)
        wo_sb = pWo.tile([P, DKT, D], BF16)
        for k in range(DKT):
            nc.sync.dma_start(out=wo_sb[:, k, :], in_=wo3[:, k, :])

        # ---------------- phase D: attention per head pair ----------------
        psum_av = tc.alloc_tile_pool(name="psum_av", bufs=2, space="PSUM")
        psum_st = tc.alloc_tile_pool(name="psum_st", bufs=2, space="PSUM")
        pEx = tc.alloc_tile_pool(name="pEx", bufs=4)
        pRc = tc.alloc_tile_pool(name="pRc", bufs=4)

        # Softmax tail note: the reciprocal and final multiply wait on DMA
        # round trips, so they are DEFERRED into the middle of the NEXT
        # pair's km loop -- otherwise they head-of-line block the DVE queue
        # (the next pair's exp tiles queue behind them), which transitively
        # stalls the PE and drops it out of its top p-state.
        prev_rc, prev_mul = [], []

        for p in range(NPAIR):
            hA, hB = 2 * p, 2 * p + 1
            avA = psum_av.tile([65, QL], F32, tag="av")
            avB = psum_av.tile([65, QL], F32, tag="av")

            def emit_av(k, exA, exB, avA=avA, avB=avB, hA=hA, hB=hB):
                stF, spF = k == 0, k == KMT - 1
                mm(avA[:, 0:512], vh[:, k, hA, 0:65], exA[:, 0:512],
                   start=stF, stop=spF, skip_group_check=True)
                mm(avA[:, 512:1024], vh[:, k, hA, 0:65], exA[:, 512:1024],
                   start=stF, stop=spF, skip_group_check=True)
                mm(avB[:, 0:512], vh[:, k, hB, 0:65], exB[:, 0:512],
                   start=stF, stop=spF, skip_group_check=True)
                mm(avB[:, 512:1024], vh[:, k, hB, 0:65], exB[:, 512:1024],
                   start=stF, stop=spF, skip_group_check=True)

            def emit_exp(st_ps, ex_t, idx):
                if idx % 8 in DVE_EXP:
                    nc.vector.tensor_scalar(
                        out=ex_t.bitcast(I16), in0=st_ps,
                        scalar1=A16, scalar2=B16, op0=MULT, op1=ADD)
                else:
                    nc.scalar.activation(ex_t, st_ps, EXP, scale=0.125)

            prev = None
            for km in range(KMT):
                if km == 5:
                    for f in prev_rc:
                        f()
                    prev_rc = []
                if km == 11:
                    for f in prev_mul:
                        f()
                    prev_mul = []
                kslA = kh[0:64, p, km * P : (km + 1) * P]
                kslB = kh[64:128, p, km * P : (km + 1) * P]
                stA = psum_st.tile([P, QL], F32, tag="st")
                stB = psum_st.tile([P, QL], F32, tag="st")
                mm(stA[:, 0:512], kslA, qh[0:64, p, 0:512])
                mm(stA[:, 512:1024], kslA, qh[0:64, p, 512:1024])
                mm(stB[:, 0:512], kslB, qh[64:128, p, 0:512])
                mm(stB[:, 512:1024], kslB, qh[64:128, p, 512:1024])
                exA = pEx.tile([P, QL], BF16, tag="ex")
                exB = pEx.tile([P, QL], BF16, tag="ex")
                emit_exp(stA, exA, 2 * km)
                emit_exp(stB, exB, 2 * km + 1)
                if prev is not None:
                    emit_av(km - 1, *prev)
                prev = (exA, exB)
            emit_av(KMT - 1, *prev)

            # tail stage 1 (now): rows 0-63 = attn_unnorm^T, row 64 = l.
            # Evict both immediately so the av psum frees; ship l to DRAM
            # and reload it reshaped [128, 8] (a [1, N] DVE reciprocal is
            # serial at ~6.4ns/elem; the reshaped one is ~60x faster).
            for head, av in ((0, avA), (1, avB)):
                rows = slice(0, 64) if head == 0 else slice(64, 128)
                h = 2 * p + head
                lrow = pRc.tile([1, QL], F32, tag="lrow")
                nc.vector.tensor_copy(out=lrow, in_=av[64:65, :])
                nc.vector.tensor_copy(out=attn[rows, p, :], in_=av[0:64, :])
                nc.sync.dma_start(out=ld[h : h + 1, :], in_=lrow)
                l128 = pRc.tile([P, QMT], F32, tag="l128")
                nc.sync.dma_start(out=l128, in_=ld3[h])
                rc_holder = []

                def run_rc(h=h, l128=l128, rc_holder=rc_holder):
                    rc128 = pRc.tile([P, QMT], F32, tag="rc128")
                    nc.vector.reciprocal(rc128, l128)
                    rcrow = rcd[h : h + 1, :]
                    rc3 = rcrow.rearrange("o (c p) -> (o p) c", p=P)
                    nc.sync.dma_start(out=rc3, in_=rc128)
                    rc_bc = pRc.tile([P, QL], F32, tag="rcbc")
                    rc_bcast = bass.AP(
                        tensor=rcrow.tensor, offset=rcrow.offset,
                        ap=[[0, P]] + [list(x) for x in rcrow.ap[1:]])
                    nc.sync.dma_start(out=rc_bc, in_=rc_bcast)
                    rc_holder.append(rc_bc)

                def run_mul(rows=rows, p=p, rc_holder=rc_holder):
                    nc.vector.tensor_mul(attn[rows, p, :], attn[rows, p, :],
                                         rc_holder[0][rows, :])

                prev_rc.append(run_rc)
                prev_mul.append(run_mul)

        # last pair's tail has no following pair to hide in
        for f in prev_rc:
            f()
        for f in prev_mul:
            f()
        pRc.release()
        pEx.release()
        psum_st.release()
        psum_av.release()

        # ---------------- phase E: output projection ----------------
        psum_E = tc.alloc_tile_pool(name="psum_E", bufs=8, space="PSUM")
        pOut = tc.alloc_tile_pool(name="pOut", bufs=2)

        # two mt-groups; within each, k=0..6 accumulate for all four mts
        # first, then k=7 + eviction -- so the PE has real work while the
        # last pair's deferred normalize (whose writes k=7 reads) lands.
        for g in range(2):
            mts = range(4 * g, 4 * g + 4)
            pss = {}
            for mt in mts:
                ps0 = psum_E.tile([P, 512], F32, tag="eps")
                ps1 = psum_E.tile([P, 512], F32, tag="eps")
                pss[mt] = (ps0, ps1)
                for k in range(DKT - 1):
                    st = k == 0
                    a_sl = attnp[k][:, mt * P : (mt + 1) * P]
                    mm(ps0, a_sl, wo_sb[:, k, 0:512],
                       start=st, stop=False, skip_group_check=True)
                    mm(ps1, a_sl, wo_sb[:, k, 512:1024],
                       start=st, stop=False, skip_group_check=True)
            for mt in mts:
                ps0, ps1 = pss[mt]
                a_sl = attnp[DKT - 1][:, mt * P : (mt + 1) * P]
                mm(ps0, a_sl, wo_sb[:, DKT - 1, 0:512],
                   start=False, stop=True, skip_group_check=True)
                mm(ps1, a_sl, wo_sb[:, DKT - 1, 512:1024],
                   start=False, stop=True, skip_group_check=True)
                o_sb = pOut.tile([P, D], F32, tag="osb")
                nc.vector.tensor_add(out=o_sb[:, 0:512], in0=ps0,
                                     in1=bias_sb[:, 0:512])
                nc.vector.tensor_add(out=o_sb[:, 512:1024], in0=ps1,
                                     in1=bias_sb[:, 512:1024])
                nc.sync.dma_start(out=out3[:, mt, :], in_=o_sb)

        pOut.release()
        psum_E.release()
        pWo.release()
        pers.release()

    nc.compile()
    return nc


_NC = None


def _get_nc():
    global _NC
    if _NC is None:
        _NC = _build_nc()
    return _NC


def kernel(Q, K, V, mask, W_q, W_k, W_v, W_o_w, W_o_b):
    global LAST_RESULTS
    BF = ml_dtypes.bfloat16
    Q = np.asarray(Q, dtype=np.float32)
    K = np.asarray(K, dtype=np.float32)
    V = np.asarray(V, dtype=np.float32)
    W_q = np.asarray(W_q, dtype=np.float32)
    W_k = np.asarray(W_k, dtype=np.float32)
    W_v = np.asarray(W_v, dtype=np.float32)
    W_o_w = np.asarray(W_o_w, dtype=np.float32)
    W_o_b = np.asarray(W_o_b, dtype=np.float32)

    # weight shards (shared by all cores); host-side transpose + bf16 cast is
    # data movement only
    wq_h = np.ascontiguousarray(W_q.transpose(1, 0, 2).reshape(D, D).astype(BF))
    wk_h = np.ascontiguousarray(W_k.transpose(1, 0, 2).reshape(D, D).astype(BF))
    wv_h = np.ascontiguousarray(W_v.transpose(1, 0, 2).reshape(D, D).astype(BF))
    wo_h = np.ascontiguousarray(W_o_w.T.astype(BF))
    wob_h = np.ascontiguousarray(W_o_b.reshape(1, D))

    in_maps = []
    for c in range(8):
        b, qs = c // 2, (c % 2) * QL
        in_maps.append({
            "qt": np.ascontiguousarray(Q[b, qs : qs + QL, :].T.astype(BF)),
            "ktd": np.ascontiguousarray(K[b].T.astype(BF)),
            "vtd": np.ascontiguousarray(V[b].T.astype(BF)),
            "wq": wq_h,
            "wk": wk_h,
            "wv": wv_h,
            "wo": wo_h,
            "wob": wob_h,
        })

    nc = _get_nc()
    res = run_bass_kernel_spmd(nc, in_maps, core_ids=list(range(8)))
    LAST_RESULTS = res

    out = np.empty((4, 2 * QL, D), dtype=np.float32)
    for c in range(8):
        b, qs = c // 2, (c % 2) * QL
        out[b, qs : qs + QL, :] = res.results[c]["out"]
    return out
